# revision 9
# baseline (speedup 1.0000x reference)
"""TRN2 Bass/Tile kernel for nn_DotProductAttention (softmax over the QUERY axis).

reference:
    scores  = einsum('bqd,bkd->bqk', q, k) / sqrt(64)
    weights = softmax(scores, axis=1)          # over q, NOT k!
    out     = einsum('bqk,bkd->bqd', weights, v)

Transposed-score formulation: T = K @ Q^T (shape [k, q]) so the softmax axis
(q) is the free axis; the normalizer Z[k] folds into V (V' = V / Z).

Fully fused single-phase pipeline:
  - PSUM carries two [128,1024] half-score tiles (double buffered) PLUS the
    [128,2048] AV accumulator `pot` resident for the whole kernel, so the AV
    matmuls interleave with the score/exp stream chunk by chunk (2-chunk lag)
    instead of forming a serial phase at the end.
  - exp runs on ACT at FD=1024 (the pipeline pacer); the softmax normalizer
    reductions are split between the DVE (tensor_reduce) and the otherwise
    idle GPSIMD (tensor_scalar accum_out trick) so neither engine exceeds the
    ACT stream time.
  - q/k arrive as bf16 via SWDGE cast-DMAs (no f32 staging, no DVE converts);
    KT chunks 4..15 via a bf16 DRAM-roundtrip xbar transpose off the critical
    path; chunks 0..3 via PE transposes.
  - Output: pot -> OT (bf16, ACT copies) -> PE transposes -> O_all (f32) ->
    DMA, pipelined per 512-column quad.

Sharding: B=16 batches, data-parallel over 8 cores => 2 batches per core,
packed into the two 64-partition halves of [128, *] tiles.
"""

import math
from contextlib import ExitStack

import numpy as np

import concourse.bass as bass  # noqa: F401
import concourse.mybir as mybir
import concourse.tile as tile
from bass_rust import add_dep_helper  # noqa: F401
from concourse import bacc, bass_utils
from concourse.masks import make_identity

FP32 = mybir.dt.float32
BF16 = mybir.dt.bfloat16

N_CORES = 8
B_FULL = 16
BPC = B_FULL // N_CORES  # batches per core = 2
S = 2048
D = 64
NCH = S // 128  # 16 key chunks of 128
SCALE = 1.0 / math.sqrt(D)

AV_LAG = 2  # chunks the AV accumulation trails the score/exp stream
N_ACC_TAIL = 1  # trailing chunks whose Z comes from ACT accum_out (latency)


def emit_kernel(ctx: ExitStack, tc, q, k, v, o):
    """Emit the per-core Tile program. q/k/v/o are DRAM APs of [BPC, S, D] f32."""
    nc = tc.nc

    const_pool = ctx.enter_context(tc.tile_pool(name="const", bufs=1))
    big = ctx.enter_context(tc.tile_pool(name="big", bufs=1))
    dram = ctx.enter_context(tc.tile_pool(name="dram", bufs=1, space="DRAM"))
    # PSUM budget (16KB/partition): tag "sc" = two [128,1024] f32 half-score
    # tiles (8KB), tag "pot" = the [128,2048] f32 AV accumulator (8KB).
    ps = ctx.enter_context(tc.tile_pool(name="ps", bufs=1, space="PSUM"))

    identb = const_pool.tile([128, 128], BF16, name="identb")
    make_identity(nc, identb)
    zw = const_pool.tile([128, 128], BF16, name="zw")
    nc.vector.memset(zw[:], 0.0)

    # (b,d)-packed transposed operands: partitions 0:64 = batch0 d, 64:128 = b1 d.
    QT = big.tile([128, S], BF16, name="QT")
    KT = big.tile([128, S], BF16, name="KT")
    # staged (s, (m b d)) bf16 copies straight from cast-DMA
    qbf = big.tile([128, S], BF16, name="qbf")
    kbf = big.tile([128, S], BF16, name="kbf")
    kbf_dram = dram.tile([S, 128], BF16, name="kbf_dram")
    # V chunks [128 k, (b, chunk, d)] f32 and Vs = V / Z (bf16)
    V = big.tile([128, BPC * NCH * D], FP32, name="V")
    Vs = big.tile([128, BPC * NCH * D], BF16, name="Vs")
    # per tile t=(b,chunk) stats columns: [z, 1/z] (+2 scratch for accum halves)
    stats = big.tile([128, BPC * NCH * 4], FP32, name="stats")
    # E[t*S :+ S] = exp(scores/sqrt(D)): [128 k, 2048 q] bf16, fully resident
    E = big.tile([128, BPC * NCH * S], BF16, name="E")
    # fold scratch for the two-stage Z reduction (DVE tensor_tensor at 2x)
    fold1 = big.tile([128, 1024], BF16, name="fold1")
    fold2 = big.tile([128, 512], BF16, name="fold2")
    # tiny initialized input for the ACT table-preload dummy
    zscr = big.tile([128, 8], BF16, name="zscr")
    # O^T staging ((b,d) packed on partitions, q on free), bf16
    OT = big.tile([128, S], BF16, name="OT")
    # O in natural layout: column chunk m holds [q-tile m, (b d)], f32
    O_all = big.tile([128, S], FP32, name="O_all")

    # The AV accumulator lives for the whole kernel.
    pot = ps.tile([128, S], FP32, tag="pot", name="pot")

    # ---- ACT table preload: dummy exp while DMAs run --------------------
    nc.scalar.activation(
        zscr[:, 0:1], zw[:, 0:1], mybir.ActivationFunctionType.Exp
    )

    # ---------------- loads: SWDGE cast-DMAs for q/k, sync f32 for v ------
    # GPSIMD queue order: k quarter 0 (chunks 0..3), q quarters 0..3, k
    # quarters 1..3 — all DMAs strictly before the gpsimd Z-reduces.
    QRT = NCH // 4  # chunks per quarter
    kq_dma = {}
    qq_dma = {}

    def cast_load(dst, src, Q, b):
        ssl = slice(Q * QRT * 128, (Q + 1) * QRT * 128)
        return nc.gpsimd.dma_start(
            dst[:, ssl].rearrange("p (m b d) -> p m b d", m=QRT, b=BPC, d=D)[
                :, :, b, :
            ],
            src[b, ssl, :].rearrange("(m p) d -> p m d", p=128),
        )

    for b in range(BPC):
        kq_dma[(0, b)] = cast_load(kbf, k, 0, b)
    for Q in range(4):
        for b in range(BPC):
            qq_dma[(Q, b)] = cast_load(qbf, q, Q, b)
    for Q in range(1, 4):
        for b in range(BPC):
            kq_dma[(Q, b)] = cast_load(kbf, k, Q, b)

    # v quarters (f32, HWDGE sync ring)
    for Q in range(4):
        for b in range(BPC):
            nc.sync.dma_start(
                V[:].rearrange("p (b m d) -> p b m d", b=BPC, m=NCH)[
                    :, b, Q * QRT : (Q + 1) * QRT, :
                ],
                v[b, Q * QRT * 128 : (Q + 1) * QRT * 128, :].rearrange(
                    "(m p) d -> p m d", p=128
                ),
            )

    # k roundtrip for chunks 4..15: SBUF bf16 -> DRAM -> xbar transpose
    nc.sync.dma_start(
        kbf_dram[512:1024, :].rearrange("(m p) c -> p m c", p=128),
        kbf[:, 512:1024].rearrange("p (m c) -> p m c", m=4),
    )
    nc.sync.dma_start_transpose(out=KT[:, 512:1024], in_=kbf_dram[512:1024, :])
    nc.sync.dma_start(
        kbf_dram[1024:S, :].rearrange("(m p) c -> p m c", p=128),
        kbf[:, 1024:S].rearrange("p (m c) -> p m c", m=8),
    )
    nc.sync.dma_start_transpose(out=KT[:, 1024:S], in_=kbf_dram[1024:S, :])

    # ---------------- PE transposes: q chunks 0..15, k chunks 0..3 --------
    def pe_transpose(src_bf, dst, m, drain_engine):
        pt = ps.tile([128, 128], BF16, tag="sc", bufs=2, name=f"pt_{m}")
        nc.tensor.transpose(pt[:], src_bf[:, m * 128 : (m + 1) * 128], identb[:])
        if drain_engine == "v":
            nc.vector.tensor_copy(dst[:, m * 128 : (m + 1) * 128], pt[:])
        else:
            nc.scalar.copy(dst[:, m * 128 : (m + 1) * 128], pt[:])

    # quarter 0 of q, then k chunks 0..1, then the rest interleaved so the
    # first scores fire as early as possible
    for m in range(4):
        pe_transpose(qbf, QT, m, "v")
    pe_transpose(kbf, KT, 0, "v")
    pe_transpose(kbf, KT, 1, "v")
    for m in range(4, 8):
        pe_transpose(qbf, QT, m, "v")
    pe_transpose(kbf, KT, 2, "v")
    pe_transpose(kbf, KT, 3, "v")
    for m in range(8, NCH):
        pe_transpose(qbf, QT, m, "v")

    # ---------------- bank-opening zero matmuls for pot -------------------
    # Zero weights write 0 across all 128 partitions of each 512-col region
    # and set has_written, so the partition-sliced AV matmuls accumulate with
    # start=False regardless of bank-clear scoping.
    zmm = []
    for j in range(4):
        zmm.append(
            nc.tensor.matmul(
                pot[:, j * 512 : (j + 1) * 512],
                lhsT=zw[:],
                rhs=qbf[:, 0:512],
                start=True,
                stop=False,
                skip_group_check=True,
            )
        )

    # ---------------- fused main loop -------------------------------------
    # t = 2*i + b indexes the 32 (chunk, batch) score tiles; each is split
    # into two [128,1024] PSUM half-tiles (tag "sc", double-buffered).
    def new_half_tile(i, b, h):
        return ps.tile([128, 1024], FP32, tag="sc", bufs=2, name=f"sc{i}_{b}_{h}")

    def emit_scores_j(sct, i, b, h, jj):
        j = 2 * h + jj
        nc.tensor.matmul(
            sct[:, jj * 512 : (jj + 1) * 512],
            lhsT=KT[b * 64 : (b + 1) * 64, i * 128 : (i + 1) * 128],
            rhs=QT[b * 64 : (b + 1) * 64, j * 512 : (j + 1) * 512],
            start=True,
            stop=True,
        )

    def emit_exp(i, b, h, sct, accum):
        t = 2 * i + b
        eb = (b * NCH + i) * S + h * 1024
        if accum:
            nc.scalar.activation(
                E[:, eb : eb + 1024],
                sct[:],
                mybir.ActivationFunctionType.Exp,
                scale=SCALE,
                accum_out=stats[:, 4 * t + 2 + h : 4 * t + 3 + h],
            )
        else:
            nc.scalar.activation(
                E[:, eb : eb + 1024],
                sct[:],
                mybir.ActivationFunctionType.Exp,
                scale=SCALE,
            )

    def emit_z(i, b):
        """Normalizer Z for tile t, then 1/Z and Vs = V / Z."""
        t = 2 * i + b
        eb = (b * NCH + i) * S
        zc = stats[:, 4 * t : 4 * t + 1]
        if i >= NCH - N_ACC_TAIL:
            # Z halves came from ACT accum_out; just add them.
            nc.vector.tensor_add(
                zc, stats[:, 4 * t + 2 : 4 * t + 3], stats[:, 4 * t + 3 : 4 * t + 4]
            )
        else:
            # two 2x-rate bf16 folds then a short 1x reduce: 1.72us/tile
            # instead of 2.29us for a straight FD=2048 tensor_reduce
            nc.vector.tensor_add(fold1[:], E[:, eb : eb + 1024], E[:, eb + 1024 : eb + S])
            nc.vector.tensor_add(fold2[:], fold1[:, 0:512], fold1[:, 512:1024])
            nc.vector.tensor_reduce(
                zc,
                fold2[:],
                mybir.AxisListType.X,
                mybir.AluOpType.add,
            )
        vb = (b * NCH + i) * D
        nc.vector.reciprocal(stats[:, 4 * t + 1 : 4 * t + 2], zc)
        nc.vector.tensor_scalar_mul(
            Vs[:, vb : vb + D], V[:, vb : vb + D], stats[:, 4 * t + 1 : 4 * t + 2]
        )

    def emit_av_pair(i):
        """AV accumulation for chunk i, both batches interleaved (col strips)."""
        for j in range(4):
            for b in range(BPC):
                vb = (b * NCH + i) * D
                eb = (b * NCH + i) * S
                mm = nc.tensor.matmul(
                    pot[b * 64 : (b + 1) * 64, j * 512 : (j + 1) * 512],
                    lhsT=Vs[:, vb : vb + D],
                    rhs=E[:, eb + j * 512 : eb + (j + 1) * 512],
                    start=False,
                    stop=(i == NCH - 1 and b == BPC - 1),
                    skip_group_check=True,
                )
                if i == 0:
                    add_dep_helper(
                        mm.ins,
                        zmm[j].ins,
                        sync=False,
                        reason="AV accumulation after bank-opening zero matmul",
                    )

    for i in range(NCH):
        accum = i >= NCH - N_ACC_TAIL
        for h in range(2):
            # b-interleaved emission so the two batches' matmuls co-run in
            # disjoint PE row strips
            sA = new_half_tile(i, 0, h)
            sB = new_half_tile(i, 1, h)
            for jj in range(2):
                emit_scores_j(sA, i, 0, h, jj)
                emit_scores_j(sB, i, 1, h, jj)
            emit_exp(i, 0, h, sA, accum)
            emit_exp(i, 1, h, sB, accum)
        emit_z(i, 0)
        emit_z(i, 1)
        if i >= AV_LAG:
            emit_av_pair(i - AV_LAG)
    for i in range(NCH - AV_LAG, NCH):
        emit_av_pair(i)

    # ---------------- tail: unpack, transpose, store ----------------------
    o_view = O_all[:].rearrange("p (m b d) -> p m b d", m=NCH, b=BPC, d=D)
    for j in range(4):
        nc.scalar.copy(OT[:, j * 512 : (j + 1) * 512], pot[:, j * 512 : (j + 1) * 512])
        for m in range(4 * j, 4 * j + 4):
            ptc = ps.tile([128, 128], BF16, tag="sc", bufs=2, name=f"ptc_{m}")
            nc.tensor.transpose(ptc[:], OT[:, m * 128 : (m + 1) * 128], identb[:])
            if m % 2 == 0:
                nc.vector.tensor_copy(O_all[:, m * 128 : (m + 1) * 128], ptc[:])
            else:
                nc.scalar.copy(O_all[:, m * 128 : (m + 1) * 128], ptc[:])
        for b in range(BPC):
            nc.sync.dma_start(
                o[b, 4 * j * 128 : (4 * j + 4) * 128, :].rearrange(
                    "(m p) d -> p m d", p=128
                ),
                o_view[:, 4 * j : 4 * j + 4, b, :],
            )


_CACHE: dict = {}


def build_program():
    if "nc" in _CACHE:
        return _CACHE["nc"]
    nc = bacc.Bacc("TRN2", target_bir_lowering=False, debug=False)
    q = nc.dram_tensor("q", [BPC, S, D], FP32, kind="ExternalInput").ap()
    k = nc.dram_tensor("k", [BPC, S, D], FP32, kind="ExternalInput").ap()
    v = nc.dram_tensor("v", [BPC, S, D], FP32, kind="ExternalInput").ap()
    o = nc.dram_tensor("o", [BPC, S, D], FP32, kind="ExternalOutput").ap()
    with tile.TileContext(nc) as tc:
        with ExitStack() as ctx:
            emit_kernel(ctx, tc, q, k, v, o)
    nc.compile()
    _CACHE["nc"] = nc
    return nc


def make_in_maps(q, k, v):
    q = np.ascontiguousarray(q, dtype=np.float32)
    k = np.ascontiguousarray(k, dtype=np.float32)
    v = np.ascontiguousarray(v, dtype=np.float32)
    assert q.shape == (B_FULL, S, D), q.shape
    return [
        {
            "q": np.ascontiguousarray(q[c * BPC : (c + 1) * BPC]),
            "k": np.ascontiguousarray(k[c * BPC : (c + 1) * BPC]),
            "v": np.ascontiguousarray(v[c * BPC : (c + 1) * BPC]),
        }
        for c in range(N_CORES)
    ]


def kernel(q, k, v, _trace=False):
    nc = build_program()
    in_maps = make_in_maps(q, k, v)
    res = bass_utils.run_bass_kernel_spmd(
        nc, in_maps, core_ids=list(range(N_CORES)), trace=_trace
    )
    out = np.concatenate([r["o"] for r in res.results], axis=0)
    if _trace:
        return out, res
    return out


# revision 11
# speedup vs baseline: 1.1805x; 1.1805x over previous
"""TRN2 Bass/Tile kernel for nn_DotProductAttention (softmax over the QUERY axis).

reference:
    scores  = einsum('bqd,bkd->bqk', q, k) / sqrt(64)
    weights = softmax(scores, axis=1)          # over q, NOT k!
    out     = einsum('bqk,bkd->bqd', weights, v)

Transposed-score formulation: T = K @ Q^T (shape [k, q]) so the softmax axis
(q) is the free axis; the normalizer Z[k] folds into V (V' = V / Z).

Structure (per core, 2 batches packed into the two 64-partition halves):
  - Phase A: q/k arrive bf16 via SWDGE cast-DMAs; KT chunks 0..3 by PE
    transpose, 4..15 via bf16 DRAM-roundtrip xbar transpose; v f32 on the
    sync ring.  A dense PE warmup burst (dummy transposes) fires the HAM
    SHORT window so B1 starts at 2.4 GHz.
  - B1: [128,2048] f32 score tiles double-buffered through all 8 PSUM banks;
    exp at FD=2048 on ACT (the pacer).  The softmax normalizer uses a
    fold-tree: fold1 (2048->1024) split between GPSIMD and DVE, fold2 +
    short reduce on DVE — cheaper than a straight 1x tensor_reduce and
    keeps every engine under the ACT stream time.
  - B2: dense AV accumulation (self-warms the PE), then per-j unpack (bf16),
    PE transposes, and output DMA.
"""

import math
from contextlib import ExitStack

import numpy as np

import concourse.bass as bass  # noqa: F401
import concourse.mybir as mybir
import concourse.tile as tile
from bass_rust import add_dep_helper
from concourse import bacc, bass_utils
from concourse.masks import make_identity

FP32 = mybir.dt.float32
BF16 = mybir.dt.bfloat16

N_CORES = 8
B_FULL = 16
BPC = B_FULL // N_CORES  # batches per core = 2
S = 2048
D = 64
NCH = S // 128  # 16 key chunks of 128
SCALE = 1.0 / math.sqrt(D)

N_WARMUP = 8  # dummy PE transposes to fire the HAM SHORT window early
N_GPS_FOLD = 20  # how many of the 30 fold1 ops run on gpsimd (rest on DVE)
N_ACC_TAIL = 1  # trailing chunks whose Z comes from ACT accum_out (latency)


def emit_kernel(ctx: ExitStack, tc, q, k, v, o):
    """Emit the per-core Tile program. q/k/v/o are DRAM APs of [BPC, S, D] f32."""
    nc = tc.nc

    const_pool = ctx.enter_context(tc.tile_pool(name="const", bufs=1))
    big = ctx.enter_context(tc.tile_pool(name="big", bufs=1))
    dram = ctx.enter_context(tc.tile_pool(name="dram", bufs=1, space="DRAM"))
    # PSUM: tag "sc" rotates two [128,2048] f32 slots (all 8 banks) through
    # score tiles, then the AV accumulator, then the output transposes.
    ps = ctx.enter_context(tc.tile_pool(name="ps", bufs=2, space="PSUM"))

    identb = const_pool.tile([128, 128], BF16, name="identb")
    make_identity(nc, identb)
    zw = const_pool.tile([128, 128], BF16, name="zw")
    nc.vector.memset(zw[:], 0.0)

    QT = big.tile([128, S], BF16, name="QT")
    KT = big.tile([128, S], BF16, name="KT")
    qbf = big.tile([128, S], BF16, name="qbf")
    kbf = big.tile([128, S], BF16, name="kbf")
    kbf_dram = dram.tile([S, 128], BF16, name="kbf_dram")
    V = big.tile([128, BPC * NCH * D], FP32, name="V")
    Vs = big.tile([128, BPC * NCH * D], BF16, name="Vs")
    # per tile t=(b,chunk): [z, 1/z, zacc_h1, zacc_h2]
    stats = big.tile([128, BPC * NCH * 4], FP32, name="stats")
    E = big.tile([128, BPC * NCH * S], BF16, name="E")
    # fold scratch for the Z reduction tree
    fold_g = big.tile([128, 1024], BF16, name="fold_g")  # gpsimd fold1 out
    fold_d = big.tile([128, 1024], BF16, name="fold_d")  # DVE fold1 out
    fold2 = big.tile([128, 512], BF16, name="fold2")
    zscr = big.tile([128, 8], BF16, name="zscr")
    OT = big.tile([128, S], BF16, name="OT")
    O_all = big.tile([128, S], FP32, name="O_all")

    # ---- ACT table preload: dummy exp while DMAs run --------------------
    nc.scalar.activation(zscr[:, 0:1], zw[:, 0:1], mybir.ActivationFunctionType.Exp)

    # ---------------- loads: SWDGE cast-DMAs for q/k, sync f32 for v ------
    QRT = NCH // 4  # chunks per quarter

    def cast_load(dst, src, Q, b):
        ssl = slice(Q * QRT * 128, (Q + 1) * QRT * 128)
        return nc.gpsimd.dma_start(
            dst[:, ssl].rearrange("p (m b d) -> p m b d", m=QRT, b=BPC, d=D)[
                :, :, b, :
            ],
            src[b, ssl, :].rearrange("(m p) d -> p m d", p=128),
        )

    for b in range(BPC):
        cast_load(kbf, k, 0, b)
    for Q in range(4):
        for b in range(BPC):
            cast_load(qbf, q, Q, b)
    for Q in range(1, 4):
        for b in range(BPC):
            cast_load(kbf, k, Q, b)

    # v quarters (f32, HWDGE sync ring)
    for Q in range(4):
        for b in range(BPC):
            nc.sync.dma_start(
                V[:].rearrange("p (b m d) -> p b m d", b=BPC, m=NCH)[
                    :, b, Q * QRT : (Q + 1) * QRT, :
                ],
                v[b, Q * QRT * 128 : (Q + 1) * QRT * 128, :].rearrange(
                    "(m p) d -> p m d", p=128
                ),
            )

    # k roundtrip for chunks 4..15: SBUF bf16 -> DRAM -> xbar transpose
    nc.sync.dma_start(
        kbf_dram[512:1024, :].rearrange("(m p) c -> p m c", p=128),
        kbf[:, 512:1024].rearrange("p (m c) -> p m c", m=4),
    )
    nc.sync.dma_start_transpose(out=KT[:, 512:1024], in_=kbf_dram[512:1024, :])
    nc.sync.dma_start(
        kbf_dram[1024:S, :].rearrange("(m p) c -> p m c", p=128),
        kbf[:, 1024:S].rearrange("p (m c) -> p m c", m=8),
    )
    nc.sync.dma_start_transpose(out=KT[:, 1024:S], in_=kbf_dram[1024:S, :])

    # ---------------- PE warmup + transposes ------------------------------
    # Dummy back-to-back transposes keep the PE busy from t~0 so the HAM
    # SHORT window fires and B1's score matmuls run at 2.4 GHz.
    for w in range(N_WARMUP):
        ptw = ps.tile([128, 128], BF16, tag="sc", name=f"ptw_{w}")
        nc.tensor.transpose(ptw[:], identb[:], identb[:])

    def pe_transpose(src_bf, dst, m):
        pt = ps.tile([128, 128], BF16, tag="sc", name=f"pt_{m}")
        nc.tensor.transpose(pt[:], src_bf[:, m * 128 : (m + 1) * 128], identb[:])
        nc.vector.tensor_copy(dst[:, m * 128 : (m + 1) * 128], pt[:])

    for m in range(4):
        pe_transpose(qbf, QT, m)
    pe_transpose(kbf, KT, 0)
    pe_transpose(kbf, KT, 1)
    for m in range(4, 8):
        pe_transpose(qbf, QT, m)
    pe_transpose(kbf, KT, 2)
    pe_transpose(kbf, KT, 3)
    for m in range(8, NCH):
        pe_transpose(qbf, QT, m)

    # ---------------- B1: scores -> exp, double-buffered ------------------
    def emit_z(i, b):
        """Normalizer Z for tile t, then 1/Z and Vs = V / Z."""
        t = 2 * i + b
        eb = (b * NCH + i) * S
        zc = stats[:, 4 * t : 4 * t + 1]
        if i >= NCH - N_ACC_TAIL:
            nc.vector.tensor_add(
                zc, stats[:, 4 * t + 2 : 4 * t + 3], stats[:, 4 * t + 3 : 4 * t + 4]
            )
        else:
            # fold tree: 2048 -> 1024 (gpsimd or DVE) -> 512 -> reduce
            if t < N_GPS_FOLD:
                f1 = fold_g
                nc.gpsimd.tensor_tensor(
                    f1[:], E[:, eb : eb + 1024], E[:, eb + 1024 : eb + S],
                    mybir.AluOpType.add,
                )
            else:
                f1 = fold_d
                nc.vector.tensor_add(
                    f1[:], E[:, eb : eb + 1024], E[:, eb + 1024 : eb + S]
                )
            nc.vector.tensor_add(fold2[:], f1[:, 0:512], f1[:, 512:1024])
            nc.vector.tensor_reduce(
                zc, fold2[:], mybir.AxisListType.X, mybir.AluOpType.add
            )
        vb = (b * NCH + i) * D
        nc.vector.reciprocal(stats[:, 4 * t + 1 : 4 * t + 2], zc)
        nc.vector.tensor_scalar_mul(
            Vs[:, vb : vb + D], V[:, vb : vb + D], stats[:, 4 * t + 1 : 4 * t + 2]
        )

    for i in range(NCH):
        accum = i >= NCH - N_ACC_TAIL
        for b in range(BPC):
            t = 2 * i + b
            sct = ps.tile([128, S], FP32, tag="sc", name=f"sc{i}_{b}")
            for j in range(4):
                nc.tensor.matmul(
                    sct[:, j * 512 : (j + 1) * 512],
                    lhsT=KT[b * 64 : (b + 1) * 64, i * 128 : (i + 1) * 128],
                    rhs=QT[b * 64 : (b + 1) * 64, j * 512 : (j + 1) * 512],
                    start=True,
                    stop=True,
                )
                # chunk 0: exp per half so ACT starts before j3 lands
                if i == 0 and j == 1:
                    nc.scalar.activation(
                        E[:, (b * NCH + i) * S : (b * NCH + i) * S + 1024],
                        sct[:, 0:1024],
                        mybir.ActivationFunctionType.Exp,
                        scale=SCALE,
                    )
            eb = (b * NCH + i) * S
            if i == 0:
                nc.scalar.activation(
                    E[:, eb + 1024 : eb + S],
                    sct[:, 1024:S],
                    mybir.ActivationFunctionType.Exp,
                    scale=SCALE,
                )
            elif accum:
                # accum halves so Z is ready right at the end of the exps
                nc.scalar.activation(
                    E[:, eb : eb + 1024],
                    sct[:, 0:1024],
                    mybir.ActivationFunctionType.Exp,
                    scale=SCALE,
                    accum_out=stats[:, 4 * t + 2 : 4 * t + 3],
                )
                nc.scalar.activation(
                    E[:, eb + 1024 : eb + S],
                    sct[:, 1024:S],
                    mybir.ActivationFunctionType.Exp,
                    scale=SCALE,
                    accum_out=stats[:, 4 * t + 3 : 4 * t + 4],
                )
            else:
                nc.scalar.activation(
                    E[:, eb : eb + S],
                    sct[:],
                    mybir.ActivationFunctionType.Exp,
                    scale=SCALE,
                )
        emit_z(i, 0)
        emit_z(i, 1)

    # ---------------- B2: dense AV accumulation ---------------------------
    pot = ps.tile([128, S], FP32, tag="sc", name="pot")
    zmm = []
    for j in range(4):
        zmm.append(
            nc.tensor.matmul(
                pot[:, j * 512 : (j + 1) * 512],
                lhsT=zw[:],
                rhs=qbf[:, 0:512],
                start=True,
                stop=False,
                skip_group_check=True,
            )
        )
    for i in range(NCH):
        for j in range(4):
            for b in range(BPC):
                vb = (b * NCH + i) * D
                eb = (b * NCH + i) * S
                mm = nc.tensor.matmul(
                    pot[b * 64 : (b + 1) * 64, j * 512 : (j + 1) * 512],
                    lhsT=Vs[:, vb : vb + D],
                    rhs=E[:, eb + j * 512 : eb + (j + 1) * 512],
                    start=False,
                    stop=(i == NCH - 1 and b == BPC - 1),
                    skip_group_check=True,
                )
                if i == 0:
                    add_dep_helper(
                        mm.ins,
                        zmm[j].ins,
                        sync=False,
                        reason="AV accumulation after bank-opening zero matmul",
                    )

    # ---------------- tail: unpack, transpose, store ----------------------
    o_view = O_all[:].rearrange("p (m b d) -> p m b d", m=NCH, b=BPC, d=D)
    for j in range(4):
        nc.scalar.copy(OT[:, j * 512 : (j + 1) * 512], pot[:, j * 512 : (j + 1) * 512])
    for j in range(4):
        for m in range(4 * j, 4 * j + 4):
            ptc = ps.tile([128, 128], BF16, tag="sc", name=f"ptc_{m}")
            nc.tensor.transpose(ptc[:], OT[:, m * 128 : (m + 1) * 128], identb[:])
            if m % 2 == 0:
                nc.vector.tensor_copy(O_all[:, m * 128 : (m + 1) * 128], ptc[:])
            else:
                nc.scalar.copy(O_all[:, m * 128 : (m + 1) * 128], ptc[:])
        for b in range(BPC):
            nc.sync.dma_start(
                o[b, 4 * j * 128 : (4 * j + 4) * 128, :].rearrange(
                    "(m p) d -> p m d", p=128
                ),
                o_view[:, 4 * j : 4 * j + 4, b, :],
            )


_CACHE: dict = {}


def build_program():
    if "nc" in _CACHE:
        return _CACHE["nc"]
    nc = bacc.Bacc("TRN2", target_bir_lowering=False, debug=False)
    q = nc.dram_tensor("q", [BPC, S, D], FP32, kind="ExternalInput").ap()
    k = nc.dram_tensor("k", [BPC, S, D], FP32, kind="ExternalInput").ap()
    v = nc.dram_tensor("v", [BPC, S, D], FP32, kind="ExternalInput").ap()
    o = nc.dram_tensor("o", [BPC, S, D], FP32, kind="ExternalOutput").ap()
    with tile.TileContext(nc) as tc:
        with ExitStack() as ctx:
            emit_kernel(ctx, tc, q, k, v, o)
    nc.compile()
    _CACHE["nc"] = nc
    return nc


def make_in_maps(q, k, v):
    q = np.ascontiguousarray(q, dtype=np.float32)
    k = np.ascontiguousarray(k, dtype=np.float32)
    v = np.ascontiguousarray(v, dtype=np.float32)
    assert q.shape == (B_FULL, S, D), q.shape
    return [
        {
            "q": np.ascontiguousarray(q[c * BPC : (c + 1) * BPC]),
            "k": np.ascontiguousarray(k[c * BPC : (c + 1) * BPC]),
            "v": np.ascontiguousarray(v[c * BPC : (c + 1) * BPC]),
        }
        for c in range(N_CORES)
    ]


def kernel(q, k, v, _trace=False):
    nc = build_program()
    in_maps = make_in_maps(q, k, v)
    res = bass_utils.run_bass_kernel_spmd(
        nc, in_maps, core_ids=list(range(N_CORES)), trace=_trace
    )
    out = np.concatenate([r["o"] for r in res.results], axis=0)
    if _trace:
        return out, res
    return out
